# revision 4
# baseline (speedup 1.0000x reference)
"""Trainium2 Bass kernel v2 for nn_CDFLoss — sorted-rows band decomposition.

Reference semantics:
    target[i,t] = (event[i]==1) & (t >= duration[i])
    mask[i,t]   = (event[i]==1) | (t <= duration[i])
    p  = clip(F_pred, EPS, 1-EPS)
    bce = sum(mask * -(target*log(p) + (1-target)*log1p(-p))) / sum(mask)
    mono = mean(relu(F_pred[:,:-1] - F_pred[:,1:] + 0.1))
    loss = bce + 0.1*mono + 0.1*mean(biases**2)

Strategy: rows are globally sorted by (event, duration) on host and dealt
round-robin to the 8 cores, so tile k on every core draws from the same
sorted-rank window [1024k, 1024(k+1)) and has a narrow dur range
[dmin_k, dmax_k]. Outside the band [ba_k, bb_k) the BCE selector is
column-pure, so the Ln transform collapses into ACT scale/bias and the
per-element selector work (2 full-width DVE ops in v1) shrinks to 3 small
DVE ops on the band. Per pure-ev tile:
    DMA : f32->bf16 cast load [128,1024]
    DVE : mono stt (full width, unaligned -> 1x) + 3 band ops [128,W]
    ACT : ev=0: one Ln over [0, bb) (cols > dmax are fully masked out);
          ev=1: Ln over [0, ba) (scale -1: log1p(-x)) + Ln over [ba, 1024)
          (scale +1: log(x); band pre-written in place as select(t<thr,1-x,x))
    PE  : ones^T @ x colsum accumulation (mono telescope correction)
The (at most one) tile whose window straddles the ev boundary uses the v1
full-width g/q0 path. The module layout depends only on per-tile
(kind, ba, bb) metadata, computed from the actual inputs at call time and
cached; any input distribution is handled correctly (worst case: recompile).
Host combine removes masked-out Ln constants via device probes and fixes up
bf16-saturated (x -> 1.0) elements exactly as v1 did for fp16.

Measured (axon trn2, 8 cores, same-session interleaved repeat-differencing):
relative error vs reference 2.2e-5. Steady-state (r1=33,r2=129 slopes, past
the power/burst window) per-core full-workload time ~42-55 us vs the prior
fp16 full-width kernel's ~70-105 us under identical conditions (~1.9x), and
within ~10-20%% of the pure-DMA ablation (~38-42 us = 16 MiB/core at the
sustained ~400 GB/s/core HBM rate). T(repeat) is convex below repeat~33:
short bursts stream HBM at up to ~1.1 TB/s/core before settling, so
small-repeat protocols (e.g. the original r1=1,r2=33 harness) read well
below steady state. Engine budget per [128,1024] tile (measured): DMA cast
~1.14 us burst / ~1.31 us sustained; DVE mono stt ~0.88 us + band ~0.31 us;
ACT ~0.75-1.41 us; PE colsums negligible. A 3-ACT-split variant that avoids
the in-place band write measured slower (extra 352-cycle ACT overheads);
per-tile ACT instruction count is the knife edge. GPSIMD elementwise and
f32-on-chip variants are not competitive.
"""

import numpy as np
import ml_dtypes

import concourse.bacc as bacc
import concourse.mybir as mybir
from concourse import tile
from concourse.bass_utils import run_bass_kernel_spmd

F32 = mybir.dt.float32
F16 = mybir.dt.float16
BF16 = mybir.dt.bfloat16
I32 = mybir.dt.int32
OP = mybir.AluOpType
AF = mybir.ActivationFunctionType

B_FULL = 32768
T = 1024
N_CORES = 8
ROWS = B_FULL // N_CORES
TILES = ROWS // 128
MONO_MARGIN = 0.1
MONO_W = 0.1
BETA = 0.1
EPS = 1e-7
B1 = float(np.float32(np.float64(0.5) + np.float64(1e-7)))    # mixed-tile bias
BL = float(np.float32(np.float64(1.0) + np.float64(1e-7)))    # scale=-1 bias
BR = float(np.float32(1e-7))                                  # scale=+1 bias
# f32 values >= this round to bf16 1.0 (RN ties-even): 1 - 2^-9
SAT_THR = np.float32(1.0 - 2.0 ** -9)

_CACHE = {}


def compute_meta(duration, event):
    """Global sort + per-tile-window metadata (identical across cores)."""
    dur = np.asarray(duration).astype(np.int64)
    ev = np.asarray(event).astype(np.int64)
    order = np.lexsort((dur, ev))
    meta = []
    for k in range(TILES):
        w = order[1024 * k:1024 * (k + 1)]
        wd, we = dur[w], ev[w]
        dmin, dmax = int(wd.min()), int(wd.max())
        if we.min() == we.max():
            if we[0] == 1:
                ba, bb = dmin, dmax          # band [dmin, dmax)
            else:
                ba, bb = dmin + 1, dmax + 1  # band (dmin, dmax]
            ba &= ~1
            bb = min(T, (bb + 1) & ~1)
            ba = min(ba, bb)
            if bb - ba > 128:
                meta.append((2, 0, T))       # band too wide: full-width path
            else:
                meta.append((int(we[0]), ba, bb))
        else:
            meta.append((2, 0, T))           # mixed: full-width v1 path
    return order, tuple(meta)


def build_module(meta, repeat=1, dma_tags=4, bufs=2, skip_mono=False,
                 skip_band=False, skip_act=False, skip_pe=False):
    nc = bacc.Bacc("TRN2", debug=False, enable_asserts=False,
                   target_bir_lowering=False, num_devices=N_CORES)

    f_in = nc.dram_tensor("F", [ROWS, T], F32, kind="ExternalInput")
    thr_in = nc.dram_tensor("thr", [ROWS], F32, kind="ExternalInput")
    c2_in = nc.dram_tensor("c2", [ROWS], F32, kind="ExternalInput")
    s2_in = nc.dram_tensor("s2", [ROWS], F32, kind="ExternalInput")

    lnL_out = nc.dram_tensor("lnL", [128, TILES], F32, kind="ExternalOutput")
    lnR_out = nc.dram_tensor("lnR", [128, TILES], F32, kind="ExternalOutput")
    m_out = nc.dram_tensor("mono", [128, TILES], F32, kind="ExternalOutput")
    x_out = nc.dram_tensor("xsum", [1, T], F32, kind="ExternalOutput")
    p_out = nc.dram_tensor("probe", [1, 8], F32, kind="ExternalOutput")

    f_ap = f_in.ap()

    with tile.TileContext(nc) as tc:
        with (
            tc.tile_pool(name="const", bufs=1) as cpool,
            tc.tile_pool(name="x", bufs=bufs) as xpool,
            tc.tile_pool(name="work", bufs=bufs) as wpool,
            tc.tile_pool(name="psum", bufs=1, space="PSUM") as ppool,
        ):
            iota32 = cpool.tile([128, T], I32)
            nc.gpsimd.iota(iota32[:, :], pattern=[[1, T]], base=0,
                           channel_multiplier=0)
            iota16 = cpool.tile([128, T], F16)
            nc.vector.tensor_scalar_add(iota16[:, :], iota32[:, :], 0.0)

            thr_sb = cpool.tile([128, TILES], F32)
            c2_sb = cpool.tile([128, TILES], F32)
            s2_sb = cpool.tile([128, TILES], F32)
            nc.sync.dma_start(thr_sb[:, :],
                              thr_in.ap().rearrange("(k p) -> p k", p=128))
            nc.sync.dma_start(c2_sb[:, :],
                              c2_in.ap().rearrange("(k p) -> p k", p=128))
            nc.sync.dma_start(s2_sb[:, :],
                              s2_in.ap().rearrange("(k p) -> p k", p=128))

            lnL_sb = cpool.tile([128, TILES], F32)
            lnR_sb = cpool.tile([128, TILES], F32)
            m_sb = cpool.tile([128, TILES], F32)
            nc.vector.memset(lnL_sb[:, :], 0.0)
            nc.vector.memset(lnR_sb[:, :], 0.0)
            nc.vector.memset(m_sb[:, :], 0.0)

            b1_sb = cpool.tile([128, 1], F32)
            nc.vector.memset(b1_sb[:, :], B1)
            bl_sb = cpool.tile([128, 1], F32)
            nc.vector.memset(bl_sb[:, :], BL)
            br_sb = cpool.tile([128, 1], F32)
            nc.vector.memset(br_sb[:, :], BR)
            ones_sb = cpool.tile([128, 1], BF16)
            nc.vector.memset(ones_sb[:, :], 1.0)

            # probes (device-exact constants for host-side correction):
            # [0] = Ln(-0.5   + BL)  -> P1  masked-out const, ev0 band
            # [1] = Ln(-1.0   + BL)  -> P2  saturated target-0, scale=-1 form
            # [2] = Ln( 0.0   + BR)  -> P3  saturated target-0, ev1 band form
            # [3] = Ln( 0*s+B1)      -> P1m masked-out const, mixed tile
            # [4] = Ln(-0.5*s1+B1)   -> P2m saturated target-0, mixed tile
            pin = cpool.tile([1, 8], F32)
            nc.vector.memset(pin[:, :], 0.0)
            nc.vector.memset(pin[:, 0:1], -0.5)
            nc.vector.memset(pin[:, 1:2], -1.0)
            nc.vector.memset(pin[:, 4:5], -0.5)
            probe_sb = cpool.tile([1, 8], F32)
            nc.vector.memset(probe_sb[:, :], 0.0)
            bl1 = cpool.tile([1, 1], F32)
            nc.vector.memset(bl1[:, :], BL)
            br1 = cpool.tile([1, 1], F32)
            nc.vector.memset(br1[:, :], BR)
            b11 = cpool.tile([1, 1], F32)
            nc.vector.memset(b11[:, :], B1)
            nc.scalar.activation(probe_sb[:, 0:2], pin[:, 0:2], AF.Ln,
                                 bias=bl1[:, :], scale=1.0)
            nc.scalar.activation(probe_sb[:, 2:3], pin[:, 2:3], AF.Ln,
                                 bias=br1[:, :], scale=1.0)
            nc.scalar.activation(probe_sb[:, 3:5], pin[:, 3:5], AF.Ln,
                                 bias=b11[:, :], scale=1.0)
            nc.sync.dma_start(p_out.ap(), probe_sb[:, :])

            ps0 = ppool.tile([1, 512], F32)
            ps1 = ppool.tile([1, 512], F32)

            # interleave ev=0 (ACT-light) and ev=1 (ACT-heavy, 2 instrs)
            # tiles so neither engine sees a long phase of its worst case
            k0 = [k for k in range(TILES) if meta[k][0] != 1]
            k1 = [k for k in range(TILES) if meta[k][0] == 1]
            ks = []
            for i in range(max(len(k0), len(k1))):
                if i < len(k0):
                    ks.append(k0[i])
                if i < len(k1):
                    ks.append(k1[i])

            n_mm = repeat * TILES
            mm = 0
            for k in [k for _ in range(repeat) for k in ks]:
                kind, ba, bb = meta[k]
                x = xpool.tile([128, T], BF16, tag=f"x{k % dma_tags}",
                               name=f"x{k % dma_tags}")
                nc.gpsimd.dma_start(x[:, :], f_ap[k * 128:(k + 1) * 128, :])

                # mono: sum_t max(x_t + 0.1, x_{t+1})  (reads pre-band x)
                if not skip_mono:
                    mscr = wpool.tile([128, T], BF16, tag="m")
                    nc.vector.scalar_tensor_tensor(
                        out=mscr[:, 0:T - 1], in0=x[:, 0:T - 1],
                        scalar=MONO_MARGIN, in1=x[:, 1:T],
                        op0=OP.add, op1=OP.max,
                        accum_out=m_sb[:, k:k + 1],
                    )

                # column sums of x via PE (reads pre-band x)
                if not skip_pe:
                    nc.tensor.matmul(ps0[:, :], ones_sb[:, :], x[:, 0:512],
                                     start=(mm == 0), stop=(mm == n_mm - 1))
                    nc.tensor.matmul(ps1[:, :], ones_sb[:, :], x[:, 512:T],
                                     start=(mm == 0), stop=(mm == n_mm - 1))
                mm += 1

                if kind == 2:
                    # mixed tile: v1 full-width path
                    if skip_band or skip_act:
                        continue
                    g = wpool.tile([128, T], BF16, tag="gm")
                    nc.vector.tensor_scalar(
                        out=g[:, :], in0=iota16[:, :],
                        scalar1=thr_sb[:, k:k + 1], scalar2=c2_sb[:, k:k + 1],
                        op0=OP.is_lt, op1=OP.subtract,
                    )
                    q0 = wpool.tile([128, T], BF16, tag="q0m")
                    nc.vector.scalar_tensor_tensor(
                        out=q0[:, :], in0=x[:, :], scalar=0.5,
                        in1=g[:, :], op0=OP.subtract, op1=OP.mult,
                    )
                    lnscr = wpool.tile([128, T], BF16, tag="lnL")
                    nc.scalar.activation(
                        lnscr[:, :], q0[:, :], AF.Ln,
                        bias=b1_sb[:, :], scale=s2_sb[:, k:k + 1],
                        accum_out=lnL_sb[:, k:k + 1],
                    )
                    continue

                W = bb - ba
                if W > 0 and not skip_band:
                    g = wpool.tile([128, 128], BF16, tag="g")
                    q0 = wpool.tile([128, 128], BF16, tag="q0")
                    if kind == 1:
                        # select(t<thr, 1-x, x) written in place
                        nc.vector.tensor_scalar(
                            out=g[:, 0:W], in0=iota16[:, ba:bb],
                            scalar1=thr_sb[:, k:k + 1], scalar2=0.5,
                            op0=OP.is_lt, op1=OP.subtract,
                        )
                        nc.vector.scalar_tensor_tensor(
                            out=q0[:, 0:W], in0=x[:, ba:bb], scalar=0.5,
                            in1=g[:, 0:W], op0=OP.subtract, op1=OP.mult,
                        )
                        nc.vector.tensor_scalar(
                            out=x[:, ba:bb], in0=q0[:, 0:W],
                            scalar1=-2.0, scalar2=0.5,
                            op0=OP.mult, op1=OP.add,
                        )
                    else:
                        # select(t<thr, x, 0.5) written in place
                        nc.vector.tensor_scalar(
                            out=g[:, 0:W], in0=iota16[:, ba:bb],
                            scalar1=thr_sb[:, k:k + 1], scalar2=None,
                            op0=OP.is_lt,
                        )
                        nc.vector.scalar_tensor_tensor(
                            out=q0[:, 0:W], in0=x[:, ba:bb], scalar=0.5,
                            in1=g[:, 0:W], op0=OP.subtract, op1=OP.mult,
                        )
                        nc.vector.tensor_scalar(
                            out=x[:, ba:bb], in0=q0[:, 0:W],
                            scalar1=1.0, scalar2=0.5,
                            op0=OP.mult, op1=OP.add,
                        )

                if skip_act:
                    continue
                if kind == 1:
                    if ba > 0:
                        lnscr = wpool.tile([128, T], BF16, tag="lnL")
                        nc.scalar.activation(
                            lnscr[:, 0:ba], x[:, 0:ba], AF.Ln,
                            bias=bl_sb[:, :], scale=-1.0,
                            accum_out=lnL_sb[:, k:k + 1],
                        )
                    lnscr2 = wpool.tile([128, T], BF16, tag="lnR")
                    nc.scalar.activation(
                        lnscr2[:, 0:T - ba], x[:, ba:T], AF.Ln,
                        bias=br_sb[:, :], scale=1.0,
                        accum_out=lnR_sb[:, k:k + 1],
                    )
                else:
                    lnscr = wpool.tile([128, T], BF16, tag="lnL")
                    nc.scalar.activation(
                        lnscr[:, 0:bb], x[:, 0:bb], AF.Ln,
                        bias=bl_sb[:, :], scale=-1.0,
                        accum_out=lnL_sb[:, k:k + 1],
                    )

            xsum_sb = cpool.tile([1, T], F32)
            if skip_pe:
                nc.vector.memset(xsum_sb[:, :], 0.0)
            else:
                nc.vector.tensor_scalar_add(xsum_sb[:, 0:512], ps0[:, :], 0.0)
                nc.vector.tensor_scalar_add(xsum_sb[:, 512:T], ps1[:, :], 0.0)

            nc.sync.dma_start(lnL_out.ap(), lnL_sb[:, :])
            nc.sync.dma_start(lnR_out.ap(), lnR_sb[:, :])
            nc.sync.dma_start(m_out.ap(), m_sb[:, :])
            nc.sync.dma_start(x_out.ap(), xsum_sb[:, :])

    nc.compile()
    return nc


def _get_module(meta):
    if meta not in _CACHE:
        _CACHE[meta] = build_module(meta)
    return _CACHE[meta]


def make_in_maps(F_pred, duration, event, order):
    F_pred = np.asarray(F_pred, dtype=np.float32)
    dur = np.asarray(duration).astype(np.float32)
    ev = np.asarray(event).astype(np.float32)
    thr = (dur + np.float32(0.5) - ev).astype(np.float32)
    c2 = (ev * np.float32(0.5)).astype(np.float32)
    s2 = (-(1.0 + ev)).astype(np.float32)
    in_maps = []
    for c in range(N_CORES):
        perm = order[c::N_CORES]
        in_maps.append({
            "F": np.ascontiguousarray(F_pred[perm]),
            "thr": np.ascontiguousarray(thr[perm]),
            "c2": np.ascontiguousarray(c2[perm]),
            "s2": np.ascontiguousarray(s2[perm]),
        })
    return in_maps


def combine(results, in_maps, meta, order, F_pred, biases, duration, event):
    dur_all = np.asarray(duration).astype(np.int64)
    ev_all = np.asarray(event).astype(np.int64)

    pr = results[0]["probe"].astype(np.float64).ravel()
    P1, P2, P3, P1m, P2m = pr[0], pr[1], pr[2], pr[3], pr[4]

    ln_total = np.float64(0.0)
    mono_total = np.float64(0.0)
    mask_total = np.float64(0.0)

    kinds = np.array([m[0] for m in meta])
    bas = np.array([m[1] for m in meta])
    bbs = np.array([m[2] for m in meta])

    for c in range(N_CORES):
        perm = order[c::N_CORES]
        r = results[c]
        d = dur_all[perm]
        e = ev_all[perm]
        Fc = in_maps[c]["F"]

        ln_sum = (r["lnL"].astype(np.float64).sum()
                  + r["lnR"].astype(np.float64).sum())
        m_sum = np.float64(r["mono"].astype(np.float64).sum())
        x_sum = np.float64(r["xsum"].astype(np.float64).sum())

        # remove masked-out constants
        kd = d.reshape(TILES, 128)
        ke = e.reshape(TILES, 128)
        for k in range(TILES):
            if kinds[k] == 0:
                # ev=0 tile, ACT covered [0, bb): cols (dur, bb) gave Ln(P1)
                cnt = np.maximum(0, bbs[k] - 1 - kd[k]).sum()
                ln_sum -= np.float64(cnt) * P1
            elif kinds[k] == 2:
                # mixed tile, full width: ev=0 rows, cols > dur gave Ln(B1)
                cnt = np.where(ke[k] == 0, (T - 1) - kd[k], 0).sum()
                ln_sum -= np.float64(cnt) * P1m

        # bf16 saturation fixup: x >= SAT_THR became 1.0 on device; in
        # target-0 in-mask positions the device computed a probe constant.
        ii, tt = np.nonzero(Fc >= SAT_THR)
        if ii.size:
            di, ei = d[ii], e[ii]
            ki = ii // 128
            # target-0 in-mask: ev=1 & t<dur, or ev=0 & t<=dur
            t0 = np.where(ei == 1, tt < di, tt <= di)
            if t0.any():
                iiv, ttv, kiv = ii[t0], tt[t0], ki[t0]
                x = Fc[iiv, ttv].astype(np.float64)
                true_ln = np.log1p(-np.minimum(x, np.float64(np.float32(1.0 - EPS))))
                kindv = kinds[kiv]
                # device value per position
                in_band1 = (kindv == 1) & (ttv >= bas[kiv]) & (ttv < bbs[kiv])
                dev = np.where(in_band1, P3, np.where(kindv == 2, P2m, P2))
                ln_sum += (true_ln - dev).sum()

        ln_total += ln_sum
        mask_total += np.where(e == 1, T, d + 1).sum()

        x0 = Fc[:, 0].astype(ml_dtypes.bfloat16).astype(np.float64).sum()
        mono_total += m_sum - x_sum + x0

    bce = -ln_total / mask_total
    mono_mean = mono_total / (np.float64(B_FULL) * (T - 1))
    bias_term = np.float64(BETA) * np.mean(np.asarray(biases, np.float64) ** 2)
    loss = bce + np.float64(MONO_W) * mono_mean + bias_term
    return np.float32(loss)


def run(F_pred, biases, duration, event, **spmd_kwargs):
    order, meta = compute_meta(duration, event)
    nc = _get_module(meta)
    in_maps = make_in_maps(F_pred, duration, event, order)
    res = run_bass_kernel_spmd(nc, in_maps, core_ids=list(range(N_CORES)),
                               **spmd_kwargs)
    out = combine(res.results, in_maps, meta, order,
                  F_pred, biases, duration, event)
    return out, res


def kernel(F_pred, biases, duration, event):
    F_pred = np.asarray(F_pred)
    assert F_pred.shape == (B_FULL, T), f"unexpected shape {F_pred.shape}"
    return run(F_pred, biases, duration, event)[0]
